# revision 30
# baseline (speedup 1.0000x reference)
"""Trainium2 Bass kernel for nn_Block_78391743086814 (EViT-style block with
top-k token pruning). Data-parallel over batch: 32 samples -> 8 cores x 4.

Per sample x [785, 384]:
  h = LN1(x); qkv = h @ Wqkv^T; attn = softmax(q k^T / 8) per head (6, hd=64)
  x = x + proj(attn @ v)
  cls_attn = mean_h attn[h, 0, 1:]; stable top-K (K=549) desc
  x_cat = [x[0], x[1+idx], sum_nonkept cls_attn_j * x[1+j]]   # [551, 384]
  y = x_cat + fc2(gelu(fc1(LN2(x_cat))))

All fp32. Weights passed pre-transposed ([c_in, c_out]) from the host wrapper.
Ranking is computed as a pairwise stable rank (count of strictly-greater plus
equal-with-lower-index) so exact fp32 ties break by index like jax.lax.top_k.
"""

import numpy as np

import concourse.bacc as bacc
import concourse.bass as bass
import concourse.mybir as mybir
import concourse.tile as tile
from concourse.bass_utils import run_bass_kernel_spmd
from concourse.masks import make_identity
from concourse.tile_rust import add_dep_helper

F32 = mybir.dt.float32
I32 = mybir.dt.int32
AF = mybir.ActivationFunctionType
OP = mybir.AluOpType
AX = mybir.AxisListType

B, N, C = 32, 785, 384
H, HD = 6, 64
KEEP = 549            # ceil(0.7 * 784)
OUT_T = 551           # 1 + KEEP + 1
HID = 4 * C
EPS = 1e-5
SCALE = HD ** -0.5

NCORES = 8
SPC = B // NCORES

NT = 7                         # token tiles; last has 17 rows
LAST_T = N - 128 * (NT - 1)    # 17
TOKPAD = NT * 128              # 896
NO = 5                         # out-token tiles; last 39
LAST_O = OUT_T - 128 * (NO - 1)
OPAD = NO * 128                # 640
BIGF = float(2 ** 30)
NEG = -1.0e30

NF_N = [(0, 512), (512, N - 512)]
NF_O = [(0, 512), (512, OUT_T - 512)]


def tok_rows(t):
    return 128 if t < NT - 1 else LAST_T


def out_rows(t):
    return 128 if t < NO - 1 else LAST_O


def build(flags):
    nc = bacc.Bacc("TRN2", target_bir_lowering=False, debug=False)

    x_d = nc.dram_tensor("x", [SPC, N, C], F32, kind="ExternalInput")
    wqkv_d = nc.dram_tensor("wqkvT", [C, 3 * C], F32, kind="ExternalInput")
    wproj_d = nc.dram_tensor("wprojT", [C, C], F32, kind="ExternalInput")
    wfc1_d = nc.dram_tensor("wfc1T", [C, HID], F32, kind="ExternalInput")
    wfc2_d = nc.dram_tensor("wfc2T", [HID, C], F32, kind="ExternalInput")
    out_d = nc.dram_tensor("out", [SPC, OUT_T, C], F32, kind="ExternalOutput")
    dbg_d = None
    if flags.get("_debug"):
        dbg_d = nc.dram_tensor("dbg", [SPC, 128, 64], F32, kind="ExternalOutput")
    bias_d = {}
    for nm, sz in [("bqkv", 3 * C), ("bproj", C), ("bfc1", HID), ("bfc2", C),
                   ("n1w", C), ("n1b", C), ("n2w", C), ("n2b", C)]:
        if flags[nm]:
            bias_d[nm] = nc.dram_tensor(nm, [sz], F32, kind="ExternalInput")

    with tile.TileContext(nc) as tc:
        with (
            tc.tile_pool(name="wp", bufs=1) as wp,
            tc.tile_pool(name="pa", bufs=1) as pa,
            tc.tile_pool(name="pb", bufs=1) as pb,
            tc.tile_pool(name="ps", bufs=1, space="PSUM") as ps,
        ):
            k = Kern(nc, tc, wp, pa, pb, ps, flags, x_d, wqkv_d, wproj_d,
                     wfc1_d, wfc2_d, out_d, bias_d)
            k.dbg_d = dbg_d
            k.emit()
    nc.compile()
    return nc


class Kern:
    def __init__(self, nc, tc, wp, pa, pb, ps, flags, x_d, wqkv_d, wproj_d,
                 wfc1_d, wfc2_d, out_d, bias_d):
        self.nc = nc
        self.tc = tc
        self.wp, self.pa, self.pb, self.ps = wp, pa, pb, ps
        self.flags = flags
        self.x_d, self.out_d = x_d, out_d
        self.wqkv_d, self.wproj_d = wqkv_d, wproj_d
        self.wfc1_d, self.wfc2_d = wfc1_d, wfc2_d
        self.bias_d = bias_d
        self.out_flat = out_d[:, :, :].rearrange("s t c -> (s t) c")
        self.scatter_insts = [[] for _ in range(SPC)]
        self.fences = [None] * SPC

    # ---------------- weights ----------------
    def load_weights(self):
        nc, wp = self.nc, self.wp
        self.Wqkv = [wp.tile([128, 3 * C], F32, name=f"Wqkv{k}", tag=f"Wqkv{k}")
                     for k in range(3)]
        self.Wp = [wp.tile([128, C], F32, name=f"Wp{k}", tag=f"Wp{k}")
                   for k in range(3)]
        self.W1 = [wp.tile([128, HID], F32, name=f"W1{k}", tag=f"W1{k}")
                   for k in range(3)]
        self.W2 = [wp.tile([128, C], F32, name=f"W2{k}", tag=f"W2{k}")
                   for k in range(12)]
        for k in range(3):
            nc.sync.dma_start(out=self.Wqkv[k][:, :],
                              in_=self.wqkv_d[128 * k:128 * (k + 1), :])
            nc.sync.dma_start(out=self.Wp[k][:, :],
                              in_=self.wproj_d[128 * k:128 * (k + 1), :])
            nc.sync.dma_start(out=self.W1[k][:, :],
                              in_=self.wfc1_d[128 * k:128 * (k + 1), :])
        for k in range(12):
            nc.sync.dma_start(out=self.W2[k][:, :],
                              in_=self.wfc2_d[128 * k:128 * (k + 1), :])

        self.ident = wp.tile([128, 128], F32, name="ident", tag="ident")
        make_identity(nc, self.ident[:, :])
        self.onesr = wp.tile([1, 128], F32, name="onesr", tag="onesr")
        nc.gpsimd.memset(self.onesr[:, :], 1.0)
        self.eps_col = wp.tile([128, 1], F32, name="eps_col", tag="eps_col")
        nc.gpsimd.memset(self.eps_col[:, :], EPS)

        self.bias_col = {}
        for nm, ntile in [("bqkv", 9), ("bproj", 3), ("bfc1", 12), ("bfc2", 3)]:
            if not self.flags[nm]:
                self.bias_col[nm] = None
                continue
            t = wp.tile([128, ntile], F32, name=f"{nm}c", tag=f"{nm}c")
            for i in range(ntile):
                nc.sync.dma_start(out=t[:, i:i + 1],
                                  in_=self.bias_d[nm][128 * i:128 * (i + 1), None])
            self.bias_col[nm] = t

        # broadcast rows [128, C] for norm weights / v-part qkv bias
        self.bc = {}
        need = [(nm, self.bias_d.get(nm)) for nm in ("n1w", "n1b", "n2w", "n2b")
                if self.flags[nm]]
        if self.flags["bqkv"]:
            need.append(("vbias", None))
        for nm, src in need:
            row = wp.tile([1, C], F32, name=f"{nm}r", tag=f"{nm}r")
            if nm == "vbias":
                nc.sync.dma_start(out=row[:, :],
                                  in_=self.bias_d["bqkv"][None, 2 * C:3 * C])
            else:
                nc.sync.dma_start(out=row[:, :], in_=src[None, :])
            bcast = wp.tile([128, C], F32, name=f"{nm}bc", tag=f"{nm}bc")
            pbc = self.ps.tile([128, 512], F32, name="pbc", tag="pv", bufs=2)
            nc.tensor.matmul(out=pbc[:, :C], lhsT=self.onesr[:, :],
                             rhs=row[:, :], start=True, stop=True)
            nc.vector.tensor_copy(out=bcast[:, :], in_=pbc[:, :C])
            self.bc[nm] = bcast

    # ------------- LN stats helper -------------
    def ln_stats(self, pool, tiles, ntiles, pfx):
        """Per-token mean/rstd over C, batched into [128, 8] columns.
        Returns (r, nmr): scale and bias columns for the fused apply."""
        nc = self.nc
        sums = pool.tile([128, 8], F32, name=f"{pfx}sums", tag=f"{pfx}sums")
        sqs = pool.tile([128, 8], F32, name=f"{pfx}sqs", tag=f"{pfx}sqs")
        for t in range(ntiles):
            scr = pool.tile([128, C], F32, name=f"{pfx}scr", tag="lnscr", bufs=2)
            nc.vector.tensor_reduce(out=sums[:, t:t + 1], in_=tiles[t][:, :],
                                    axis=AX.X, op=OP.add)
            nc.scalar.activation(out=scr[:, :], in_=tiles[t][:, :],
                                 func=AF.Square, accum_out=sqs[:, t:t + 1])
        nm = pool.tile([128, 8], F32, name=f"{pfx}nm", tag=f"{pfx}nm")
        nc.vector.tensor_scalar(out=nm[:, :ntiles], in0=sums[:, :ntiles],
                                scalar1=-1.0 / C, scalar2=None, op0=OP.mult)
        var = pool.tile([128, 8], F32, name=f"{pfx}var", tag=f"{pfx}var")
        nc.vector.tensor_tensor(out=var[:, :ntiles], in0=nm[:, :ntiles],
                                in1=nm[:, :ntiles], op=OP.mult)
        nc.vector.tensor_scalar(out=sqs[:, :ntiles], in0=sqs[:, :ntiles],
                                scalar1=1.0 / C, scalar2=None, op0=OP.mult)
        nc.vector.tensor_tensor(out=var[:, :ntiles], in0=sqs[:, :ntiles],
                                in1=var[:, :ntiles], op=OP.subtract)
        # r = rsqrt(var + eps) = exp(-0.5 * ln(var + eps))
        r = pool.tile([128, 8], F32, name=f"{pfx}r", tag=f"{pfx}r")
        nc.scalar.activation(out=r[:, :ntiles], in_=var[:, :ntiles],
                             func=AF.Ln, bias=self.eps_col[:, 0:1])
        nc.scalar.activation(out=r[:, :ntiles], in_=r[:, :ntiles],
                             func=AF.Exp, scale=-0.5)
        nmr = pool.tile([128, 8], F32, name=f"{pfx}nmr", tag=f"{pfx}nmr")
        nc.vector.tensor_tensor(out=nmr[:, :ntiles], in0=nm[:, :ntiles],
                                in1=r[:, :ntiles], op=OP.mult)
        return r, nmr

    def ln_apply(self, out_t, in_t, r, nmr, t, which):
        nc = self.nc
        nc.scalar.activation(out=out_t[:, :], in_=in_t[:, :], func=AF.Identity,
                             bias=nmr[:, t:t + 1], scale=r[:, t:t + 1])
        if self.bc.get(which + "w") is not None:
            nc.vector.tensor_tensor(out=out_t[:, :], in0=out_t[:, :],
                                    in1=self.bc[which + "w"][:, :], op=OP.mult)
        if self.bc.get(which + "b") is not None:
            nc.vector.tensor_tensor(out=out_t[:, :], in0=out_t[:, :],
                                    in1=self.bc[which + "b"][:, :], op=OP.add)

    # ------------- phase A: one sample -------------
    def phase_a(self, s):
        nc, pa, ps = self.nc, self.pa, self.ps
        ident, onesr = self.ident, self.onesr

        # A1: load x token-major; X later becomes x_res in place
        X = [pa.tile([128, C], F32, name=f"X{t}", tag=f"X{t}")
             for t in range(NT)]
        nc.gpsimd.memset(X[NT - 1][:, :], 0.0)
        for t in range(NT):
            nc.sync.dma_start(out=X[t][:tok_rows(t), :],
                              in_=self.x_d[s, 128 * t:128 * t + tok_rows(t), :])

        # A2: LN1
        r1, nmr1 = self.ln_stats(pa, X, NT, "n1")
        h = []
        for t in range(NT):
            ht = pa.tile([128, C], F32, name="h_t", tag="h_t", bufs=2)
            self.ln_apply(ht, X[t], r1, nmr1, t, "n1")
            h.append(ht)

        # A3: hT = transpose(h)   3 x [128, 896]
        hT = [pa.tile([128, TOKPAD], F32, name=f"hT{c}", tag=f"hT{c}")
              for c in range(3)]
        for t in range(NT):
            for cc in range(3):
                ptr = ps.tile([128, 128], F32, name="ptr", tag="pt", bufs=2)
                nc.tensor.transpose(out=ptr[:, :],
                                    in_=h[t][:, 128 * cc:128 * (cc + 1)],
                                    identity=ident[:, :])
                nc.vector.tensor_copy(out=hT[cc][:, 128 * t:128 * (t + 1)],
                                      in_=ptr[:, :])

        # A4: q,k channel-major [6 x (128, 785)]
        qkT = [pa.tile([128, N], F32, name=f"qk{m}", tag=f"qk{m}")
               for m in range(6)]
        for m in range(6):
            for (n0, nl) in NF_N:
                pq = ps.tile([128, 512], F32, name="pq", tag="pv", bufs=2)
                for kc in range(3):
                    nc.tensor.matmul(out=pq[:, :nl],
                                     lhsT=self.Wqkv[kc][:, 128 * m:128 * (m + 1)],
                                     rhs=hT[kc][:, n0:n0 + nl],
                                     start=(kc == 0), stop=(kc == 2))
                if self.flags["bqkv"]:
                    nc.scalar.activation(out=qkT[m][:, n0:n0 + nl],
                                         in_=pq[:, :nl], func=AF.Identity,
                                         bias=self.bias_col["bqkv"][:, m:m + 1])
                else:
                    nc.scalar.copy(out=qkT[m][:, n0:n0 + nl], in_=pq[:, :nl])

        # A5: v token-major [7 x (128, 390)]; per head 64 v cols + 1 ones col
        v = [pa.tile([128, 6 * (HD + 1)], F32, name=f"v{t}", tag=f"v{t}")
             for t in range(NT)]
        for t in range(NT):
            rows = tok_rows(t)
            ones_ap = v[t][:, :].rearrange("p (h d) -> p h d", d=HD + 1)[:, :, HD:]
            if rows < 128:
                nc.gpsimd.memset(ones_ap, 0.0)
                nc.gpsimd.memset(
                    v[t][:rows, :].rearrange("p (h d) -> p h d",
                                             d=HD + 1)[:, :, HD:], 1.0)
            else:
                nc.gpsimd.memset(ones_ap, 1.0)
            pvp = ps.tile([128, 512], F32, name="pvp", tag="pv", bufs=2)
            for kc in range(3):
                nc.tensor.matmul(out=pvp[:, :C],
                                 lhsT=hT[kc][:, 128 * t:128 * (t + 1)],
                                 rhs=self.Wqkv[kc][:, 2 * C:3 * C],
                                 start=(kc == 0), stop=(kc == 2))
            src = pvp[:, :C].rearrange("p (h d) -> p h d", d=HD)
            dst = v[t][:, :].rearrange("p (h d) -> p h d", d=HD + 1)[:, :, :HD]
            nc.vector.tensor_copy(out=dst, in_=src)
            if self.flags["bqkv"]:
                nc.vector.tensor_tensor(
                    out=dst, in0=dst,
                    in1=self.bc["vbias"][:, :].rearrange("p (h d) -> p h d",
                                                         d=HD), op=OP.add)

        # A6: attention
        aout = [pa.tile([128, N], F32, name=f"aout{c}", tag=f"aout{c}")
                for c in range(3)]
        cls_e = [pa.tile([128, 8], F32, name=f"cls{t}", tag=f"cls{t}")
                 for t in range(NT)]
        cls_s = [pa.tile([128, 8], F32, name=f"clss{t}", tag=f"clss{t}")
                 for t in range(NT)]
        # invalid tail rows of the last tile contribute cls_attn = 0, which
        # ranks below every real (positive) value, so they are never kept
        nc.gpsimd.memset(cls_e[NT - 1][:, :], 0.0)
        nc.gpsimd.memset(cls_s[NT - 1][:, :], 0.0)
        rd0s = pa.tile([1, 8], F32, name="rd0s", tag="rd0s")

        for hh in range(6):
            qh = qkT[hh // 2][64 * (hh % 2):64 * (hh % 2) + 64, :]
            kh = qkT[3 + hh // 2][64 * (hh % 2):64 * (hh % 2) + 64, :]
            tp = (64 * (hh % 2), 0)
            pvacc = [ps.tile([HD + 1, nl], F32, name=f"pvacc{i}",
                             tag=f"pvacc{i}", bufs=1)
                     for i, (n0, nl) in enumerate(NF_N)]
            for t in range(NT):
                rows = tok_rows(t)
                s_ps = ps.tile([128, 512], F32, name="s_ps", tag="S", bufs=1)
                s_ps2 = ps.tile([128, N - 512], F32, name="s_ps2", tag="S2",
                                bufs=1)
                for (n0, nl), dstp in zip(NF_N, (s_ps, s_ps2)):
                    nc.tensor.matmul(out=dstp[:rows, :nl],
                                     lhsT=kh[:, 128 * t:128 * t + rows],
                                     rhs=qh[:, n0:n0 + nl],
                                     start=True, stop=True, tile_position=tp)
                eS = pa.tile([128, N], F32, name="eS", tag="eS", bufs=3)
                for (n0, nl), srcp in zip(NF_N, (s_ps, s_ps2)):
                    nc.scalar.activation(out=eS[:rows, n0:n0 + nl],
                                         in_=srcp[:rows, :nl],
                                         func=AF.Exp, scale=SCALE)
                nc.gpsimd.tensor_scalar(out=cls_e[t][:rows, hh:hh + 1],
                                        in0=eS[:rows, 0:1], scalar1=0.0,
                                        scalar2=None, op0=OP.add)
                # raw cls score column (pre-scale) for the exp refinement
                nc.vector.tensor_scalar(out=cls_s[t][:rows, hh:hh + 1],
                                        in0=s_ps[:rows, 0:1], scalar1=SCALE,
                                        scalar2=None, op0=OP.mult)
                for i, (n0, nl) in enumerate(NF_N):
                    nc.tensor.matmul(
                        out=pvacc[i][:, :],
                        lhsT=v[t][:rows, (HD + 1) * hh:(HD + 1) * (hh + 1)],
                        rhs=eS[:rows, n0:n0 + nl],
                        start=(t == 0), stop=(t == NT - 1))
            # denom -> reciprocal (in place) -> rd0s, broadcast, normalize
            dn = pa.tile([1, N], F32, name="dn", tag="dn", bufs=2)
            for i, (n0, nl) in enumerate(NF_N):
                nc.vector.tensor_copy(out=dn[0:1, n0:n0 + nl],
                                      in_=pvacc[i][HD:HD + 1, :])
            nc.vector.reciprocal(out=dn[0:1, :], in_=dn[0:1, :])
            nc.vector.tensor_copy(out=rd0s[0:1, hh:hh + 1], in_=dn[0:1, 0:1])
            for i, (n0, nl) in enumerate(NF_N):
                rb = ps.tile([64, nl], F32, name="rb", tag="pt", bufs=2)
                nc.tensor.matmul(out=rb[:, :], lhsT=onesr[0:1, 0:64],
                                 rhs=dn[0:1, n0:n0 + nl], start=True, stop=True)
                dst = aout[hh // 2][64 * (hh % 2):64 * (hh % 2) + 64,
                                    n0:n0 + nl]
                nc.scalar.copy(out=dst, in_=pvacc[i][0:HD, :])
                nc.vector.tensor_tensor(out=dst, in0=dst, in1=rb[:, :],
                                        op=OP.mult)

        # A7+A8: proj (per m-tile) -> transpose -> residual in place into X
        for m in range(3):
            pjT = pa.tile([128, TOKPAD], F32, name="pjT", tag=f"hT{m}")
            for (n0, nl) in NF_N:
                pp = ps.tile([128, 512], F32, name="pp", tag="pv", bufs=2)
                for kc in range(3):
                    nc.tensor.matmul(out=pp[:, :nl],
                                     lhsT=self.Wp[kc][:, 128 * m:128 * (m + 1)],
                                     rhs=aout[kc][:, n0:n0 + nl],
                                     start=(kc == 0), stop=(kc == 2))
                if self.flags["bproj"]:
                    nc.scalar.activation(out=pjT[:, n0:n0 + nl],
                                         in_=pp[:, :nl], func=AF.Identity,
                                         bias=self.bias_col["bproj"][:, m:m + 1])
                else:
                    nc.scalar.copy(out=pjT[:, n0:n0 + nl], in_=pp[:, :nl])
            nc.gpsimd.memset(pjT[:, N:TOKPAD], 0.0)
            for t in range(NT):
                ptr2 = ps.tile([128, 128], F32, name="ptr2", tag="pt", bufs=2)
                nc.tensor.transpose(out=ptr2[:, :],
                                    in_=pjT[:, 128 * t:128 * (t + 1)],
                                    identity=ident[:, :])
                nc.vector.tensor_tensor(
                    out=X[t][:, 128 * m:128 * (m + 1)], in0=ptr2[:, :],
                    in1=X[t][:, 128 * m:128 * (m + 1)], op=OP.add)
        # invalid tail rows of X[6] stay zero: x rows were zeroed at load and
        # pjT[:, 785:896] is zeroed before the transpose.

        # A9: cls_attn -> stable rank -> scatter
        # Newton-refine the ACT exp on the cls path: ACT exp has ~2.5e-6 rel
        # error which would flip many near-tied rankings. e1 = e0*(1+(s-ln e0))
        # cuts it to ~1e-7. Clamp before Ln so zeroed tail rows stay exactly 0.
        for t in range(NT):
            rfA = pa.tile([128, 8], F32, name="rfA", tag="rfA", bufs=2)
            rfB = pa.tile([128, 8], F32, name="rfB", tag="rfB", bufs=2)
            nc.vector.tensor_scalar(out=rfA[:, 0:6], in0=cls_e[t][:, 0:6],
                                    scalar1=1e-30, scalar2=None, op0=OP.max)
            nc.scalar.activation(out=rfA[:, 0:6], in_=rfA[:, 0:6], func=AF.Ln)
            nc.vector.tensor_tensor(out=rfB[:, 0:6], in0=cls_s[t][:, 0:6],
                                    in1=rfA[:, 0:6], op=OP.subtract)
            nc.vector.tensor_tensor(out=rfB[:, 0:6], in0=rfB[:, 0:6],
                                    in1=cls_e[t][:, 0:6], op=OP.mult)
            nc.vector.tensor_tensor(out=cls_e[t][:, 0:6], in0=cls_e[t][:, 0:6],
                                    in1=rfB[:, 0:6], op=OP.add)
        # fold the mean-over-heads 1/6 into the reciprocal row first
        nc.vector.tensor_scalar(out=rd0s[0:1, 0:6], in0=rd0s[0:1, 0:6],
                                scalar1=1.0 / 6.0, scalar2=None, op0=OP.mult)
        crd = ps.tile([128, 8], F32, name="crd", tag="pt", bufs=2)
        nc.tensor.matmul(out=crd[:, 0:6], lhsT=onesr[:, :],
                         rhs=rd0s[0:1, 0:6], start=True, stop=True)
        ca = pa.tile([128, 8], F32, name="ca", tag="ca")
        scr6 = pa.tile([128, 8], F32, name="scr6", tag="scr6")
        for t in range(NT):
            nc.vector.tensor_tensor(out=scr6[:, 0:6], in0=cls_e[t][:, 0:6],
                                    in1=crd[:, 0:6], op=OP.mult)
            nc.vector.tensor_reduce(out=ca[:, t:t + 1], in_=scr6[:, 0:6],
                                    axis=AX.X, op=OP.add)
        nc.vector.memset(ca[0:1, 0:1], NEG)

        v_row = pa.tile([1, TOKPAD], F32, name="v_row", tag="v_row")
        for t in range(NT):
            pvr = ps.tile([1, 128], F32, name="pvr", tag="pt", bufs=2)
            nc.tensor.transpose(out=pvr[:, :], in_=ca[:, t:t + 1],
                                identity=ident[:, :])
            nc.vector.tensor_copy(out=v_row[0:1, 128 * t:128 * (t + 1)],
                                  in_=pvr[:, :])
        B_sb = pa.tile([128, TOKPAD], F32, name="B_sb", tag="B_sb")
        for n0 in (0, 512):
            nl = min(512, TOKPAD - n0)
            pB = ps.tile([128, 512], F32, name="pB", tag="pv", bufs=2)
            nc.tensor.matmul(out=pB[:, :nl], lhsT=onesr[:, :],
                             rhs=v_row[0:1, n0:n0 + nl], start=True, stop=True)
            nc.vector.tensor_copy(out=B_sb[:, n0:n0 + nl], in_=pB[:, :nl])

        rank = pa.tile([128, 8], F32, name="rank", tag="rank")
        rkeq = pa.tile([128, 8], F32, name="rkeq", tag="rkeq")
        for t in range(NT):
            scr_gt = pa.tile([128, TOKPAD], F32, name="scr_gt", tag="scr_gt")
            # with accum_out, op1 is the REDUCTION op (sum of the 0/1 mask)
            nc.vector.tensor_scalar(out=scr_gt[:, :], in0=B_sb[:, :],
                                    scalar1=ca[:, t:t + 1], scalar2=0.0,
                                    op0=OP.is_gt, op1=OP.add,
                                    accum_out=rank[:, t:t + 1])
            scr_eq = pa.tile([128, TOKPAD], F32, name="scr_eq", tag="scr_eq")
            nc.gpsimd.tensor_scalar(out=scr_eq[:, :], in0=B_sb[:, :],
                                    scalar1=ca[:, t:t + 1], scalar2=None,
                                    op0=OP.is_equal)
            # keep eq counts only where j < i: iota = 128t + p - f > 0
            nc.gpsimd.affine_select(out=scr_eq[:, :], in_=scr_eq[:, :],
                                    pattern=[[-1, TOKPAD]], compare_op=OP.is_gt,
                                    fill=0.0, base=128 * t,
                                    channel_multiplier=1)
            nc.vector.tensor_reduce(out=rkeq[:, t:t + 1], in_=scr_eq[:, :],
                                    axis=AX.X, op=OP.add)
        nc.vector.tensor_tensor(out=rank[:, :NT], in0=rank[:, :NT],
                                in1=rkeq[:, :NT], op=OP.add)

        m_k = pa.tile([128, 8], F32, name="m_k", tag="m_k")
        nc.vector.tensor_scalar(out=m_k[:, :NT], in0=rank[:, :NT],
                                scalar1=float(KEEP), scalar2=None, op0=OP.is_lt)
        destf = pa.tile([128, 8], F32, name="destf", tag="destf")
        nc.vector.tensor_scalar(out=destf[:, :NT], in0=rank[:, :NT],
                                scalar1=float(s * OUT_T + 1), scalar2=None,
                                op0=OP.add)
        nc.vector.tensor_tensor(out=destf[:, :NT], in0=destf[:, :NT],
                                in1=m_k[:, :NT], op=OP.mult)
        big_t = pa.tile([128, 8], F32, name="big_t", tag="big_t")
        nc.vector.tensor_scalar(out=big_t[:, :NT], in0=m_k[:, :NT],
                                scalar1=-BIGF, scalar2=BIGF, op0=OP.mult,
                                op1=OP.add)
        nc.vector.tensor_tensor(out=destf[:, :NT], in0=destf[:, :NT],
                                in1=big_t[:, :NT], op=OP.add)
        dest_i = pa.tile([128, 8], I32, name="dest_i", tag="dest_i")
        nc.vector.tensor_copy(out=dest_i[:, :NT], in_=destf[:, :NT])

        w_nk = pa.tile([128, 8], F32, name="w_nk", tag="w_nk")
        nc.vector.tensor_scalar(out=w_nk[:, :NT], in0=ca[:, :NT], scalar1=0.0,
                                scalar2=None, op0=OP.max)
        one_m = pa.tile([128, 8], F32, name="one_m", tag="one_m")
        nc.vector.tensor_scalar(out=one_m[:, :NT], in0=m_k[:, :NT],
                                scalar1=-1.0, scalar2=1.0, op0=OP.mult,
                                op1=OP.add)
        nc.vector.tensor_tensor(out=w_nk[:, :NT], in0=w_nk[:, :NT],
                                in1=one_m[:, :NT], op=OP.mult)

        if getattr(self, "dbg_d", None) is not None:
            for col, tt in enumerate((ca, rank, rkeq, m_k, destf, w_nk)):
                nc.sync.dma_start(out=self.dbg_d[s, :, 8 * col:8 * (col + 1)],
                                  in_=tt[:, :])
            nc.sync.dma_start(out=self.dbg_d[s, :, 48:56], in_=B_sb[:, 0:8])
            nc.sync.dma_start(out=self.dbg_d[s, 0:1, 56:64],
                              in_=v_row[0:1, 0:8])
        for t in range(NT):
            inst = nc.gpsimd.indirect_dma_start(
                out=self.out_flat,
                out_offset=bass.IndirectOffsetOnAxis(ap=dest_i[:, t:t + 1],
                                                     axis=0),
                in_=X[t][:, :], in_offset=None,
                bounds_check=s * OUT_T + OUT_T - 1, oob_is_err=False)
            self.scatter_insts[s].append(inst)
        inst = nc.sync.dma_start(out=self.out_d[s, 0:1, :], in_=X[0][0:1, :])
        self.scatter_insts[s].append(inst)
        pex = ps.tile([1, C], F32, name="pex", tag="pt", bufs=2)
        for t in range(NT):
            rows = tok_rows(t)
            nc.tensor.matmul(out=pex[:, :], lhsT=w_nk[:rows, t:t + 1],
                             rhs=X[t][:rows, :], start=(t == 0),
                             stop=(t == NT - 1))
        extra_sb = pa.tile([1, C], F32, name="extra_sb", tag="extra_sb")
        nc.vector.tensor_copy(out=extra_sb[:, :], in_=pex[:, :])
        inst = nc.sync.dma_start(out=self.out_d[s, OUT_T - 1:OUT_T, :],
                                 in_=extra_sb[:, :])
        self.scatter_insts[s].append(inst)
        # single fence instruction so downstream reload DMAs only need one
        # extra wait condition (HW DMA wait slots are limited)
        fence = nc.gpsimd.nop(nofuse=True, hint=f"scatter_fence{s}")
        for si in self.scatter_insts[s]:
            add_dep_helper(fence.ins, si.ins, reason="fence after scatter")
        self.fences[s] = fence

    # ------------- phase B1: LN2 stats (exp/ln table) -------------
    def phase_b1(self, s):
        nc, pb = self.nc, self.pb
        xcs = []
        for t in range(NO):
            xc = pb.tile([128, C], F32, name="xcs", tag="xcs", bufs=2)
            if t == NO - 1:
                nc.gpsimd.memset(xc[:, :], 0.0)
            inst = nc.sync.dma_start(
                out=xc[:out_rows(t), :],
                in_=self.out_d[s, 128 * t:128 * t + out_rows(t), :])
            add_dep_helper(inst.ins, self.fences[s].ins,
                           reason="b1 reload after scatter")
            xcs.append(xc)
        r, nmr = self.ln_stats(pb, xcs, NO, f"n2_{s}")
        return r, nmr

    # ------------- phase B2: MLP (gelu table) -------------
    def phase_b2(self, s, r2, nmr2):
        nc, pb, ps = self.nc, self.pb, self.ps
        ident = self.ident
        xc = [pb.tile([128, C], F32, name=f"xc{t}", tag=f"xc{t}")
              for t in range(NO)]
        for t in range(NO):
            if t == NO - 1:
                nc.gpsimd.memset(xc[t][:, :], 0.0)
            inst = nc.sync.dma_start(
                out=xc[t][:out_rows(t), :],
                in_=self.out_d[s, 128 * t:128 * t + out_rows(t), :])
            add_dep_helper(inst.ins, self.fences[s].ins,
                           reason="b2 reload after scatter")
        l2T = [pb.tile([128, OPAD], F32, name=f"l2T{c}", tag=f"l2T{c}")
               for c in range(3)]
        for t in range(NO):
            l2 = pb.tile([128, C], F32, name="l2", tag="l2", bufs=2)
            self.ln_apply(l2, xc[t], r2, nmr2, t, "n2")
            for cc in range(3):
                ptr3 = ps.tile([128, 128], F32, name="ptr3", tag="pt", bufs=2)
                nc.tensor.transpose(out=ptr3[:, :],
                                    in_=l2[:, 128 * cc:128 * (cc + 1)],
                                    identity=ident[:, :])
                nc.vector.tensor_copy(out=l2T[cc][:, 128 * t:128 * (t + 1)],
                                      in_=ptr3[:, :])
        # fc1 -> gelu -> fc2, streamed per hidden tile, N-chunk outer
        f2T = [pb.tile([128, OPAD], F32, name=f"f2T{m}", tag=f"f2T{m}")
               for m in range(3)]
        for (n0, nl) in NF_O:
            pf2 = [ps.tile([128, nl], F32, name=f"pf2_{m}", tag=tg, bufs=1)
                   for m, tg in enumerate(("S", "S2", "pvacc0"))]
            for kc in range(12):
                pf1 = ps.tile([128, 512], F32, name="pf1", tag="pv", bufs=2)
                for k3 in range(3):
                    nc.tensor.matmul(out=pf1[:, :nl],
                                     lhsT=self.W1[k3][:, 128 * kc:128 * (kc + 1)],
                                     rhs=l2T[k3][:, n0:n0 + nl],
                                     start=(k3 == 0), stop=(k3 == 2))
                g = pb.tile([128, 512], F32, name="g", tag="g", bufs=2)
                if self.flags["bfc1"]:
                    nc.scalar.activation(out=g[:, :nl], in_=pf1[:, :nl],
                                         func=AF.Gelu,
                                         bias=self.bias_col["bfc1"][:, kc:kc + 1])
                else:
                    nc.scalar.activation(out=g[:, :nl], in_=pf1[:, :nl],
                                         func=AF.Gelu)
                for m in range(3):
                    nc.tensor.matmul(out=pf2[m][:, :],
                                     lhsT=self.W2[kc][:, 128 * m:128 * (m + 1)],
                                     rhs=g[:, :nl],
                                     start=(kc == 0), stop=(kc == 11))
            for m in range(3):
                if self.flags["bfc2"]:
                    nc.scalar.activation(out=f2T[m][:, n0:n0 + nl],
                                         in_=pf2[m][:, :], func=AF.Identity,
                                         bias=self.bias_col["bfc2"][:, m:m + 1])
                else:
                    nc.scalar.copy(out=f2T[m][:, n0:n0 + nl], in_=pf2[m][:, :])
        for t in range(NO):
            y_t = pb.tile([128, C], F32, name="y_t", tag="y_t", bufs=2)
            for cc in range(3):
                ptr4 = ps.tile([128, 128], F32, name="ptr4", tag="pt", bufs=2)
                nc.tensor.transpose(out=ptr4[:, :],
                                    in_=f2T[cc][:, 128 * t:128 * (t + 1)],
                                    identity=ident[:, :])
                nc.vector.tensor_tensor(out=y_t[:, 128 * cc:128 * (cc + 1)],
                                        in0=ptr4[:, :],
                                        in1=xc[t][:, 128 * cc:128 * (cc + 1)],
                                        op=OP.add)
            nc.sync.dma_start(out=self.out_d[s, 128 * t:128 * t + out_rows(t), :],
                              in_=y_t[:out_rows(t), :])

    def emit(self):
        self.load_weights()
        for s in range(SPC):
            self.phase_a(s)
        stats = [self.phase_b1(s) for s in range(SPC)]
        for s in range(SPC):
            self.phase_b2(s, *stats[s])


_CACHE = {}


def kernel(x, norm1_w, norm1_b, qkv_w, qkv_b, proj_w, proj_b,
           norm2_w, norm2_b, fc1_w, fc1_b, fc2_w, fc2_b,
           _trace=False, _tmpdir=None):
    x = np.ascontiguousarray(np.asarray(x, dtype=np.float32))
    assert x.shape == (B, N, C), x.shape

    flags = {
        "bqkv": bool(np.any(np.asarray(qkv_b))),
        "bproj": bool(np.any(np.asarray(proj_b))),
        "bfc1": bool(np.any(np.asarray(fc1_b))),
        "bfc2": bool(np.any(np.asarray(fc2_b))),
        "n1w": not bool(np.all(np.asarray(norm1_w) == 1.0)),
        "n1b": bool(np.any(np.asarray(norm1_b))),
        "n2w": not bool(np.all(np.asarray(norm2_w) == 1.0)),
        "n2b": bool(np.any(np.asarray(norm2_b))),
    }
    key = tuple(sorted(flags.items()))
    if key not in _CACHE:
        _CACHE[key] = build(flags)
    nc = _CACHE[key]

    base = {
        "wqkvT": np.ascontiguousarray(np.asarray(qkv_w, np.float32).T),
        "wprojT": np.ascontiguousarray(np.asarray(proj_w, np.float32).T),
        "wfc1T": np.ascontiguousarray(np.asarray(fc1_w, np.float32).T),
        "wfc2T": np.ascontiguousarray(np.asarray(fc2_w, np.float32).T),
    }
    opt = {"bqkv": qkv_b, "bproj": proj_b, "bfc1": fc1_b, "bfc2": fc2_b,
           "n1w": norm1_w, "n1b": norm1_b, "n2w": norm2_w, "n2b": norm2_b}
    for nm, arr in opt.items():
        if flags[nm]:
            base[nm] = np.ascontiguousarray(np.asarray(arr, np.float32))

    in_maps = []
    for i in range(NCORES):
        m = dict(base)
        m["x"] = np.ascontiguousarray(x[SPC * i:SPC * (i + 1)])
        in_maps.append(m)

    kw = {}
    if _trace:
        kw["trace"] = True
    if _tmpdir:
        kw["tmpdir"] = _tmpdir
    res = run_bass_kernel_spmd(nc, in_maps, core_ids=list(range(NCORES)), **kw)
    out = np.concatenate([r["out"] for r in res.results], axis=0)
    kernel._last_result = res
    return out
